# revision 12
# baseline (speedup 1.0000x reference)
"""ConceptContrastiveLoss Trainium2 kernel (8-core SPMD, batch-parallel).

Takes FULL inputs expert_concepts/violator_concepts [256, 2048, 128] f32,
returns the scalar loss. Internally shards the batch dim across 8 cores.

Bulk phase (memory-bound, ~64 MiB HBM reads per core): each batch item
[2048, 128] is DMA'd as one contiguous 1 MiB transfer into SBUF
[128 partitions x 2048] (16 seq rows per partition), tree-halved on
VectorE (exact fp32 adds) down to [128, 128], then one fp32 ones-matmul
on TensorE folds the partitions into a D-major centroid column
accumulated in PSUM (16 batch columns per PSUM bank).

Everything else hides under the bulk DMA stream:
- Violator batches stream FIRST.  As soon as the 32 local violator
  centroids exist (~half-time), they are AllGathered (tiny, [128,32]),
  reloaded as the full [D, 256] CtV, and the violator clustering term is
  computed locally via the exact identity
      sum_{i<j} ||c_i - c_j||^2 = B * sum_i ||c_i||^2 - ||sum_i c_i||^2
  (no pairwise matrix needed).  The ||v_j||^2 row for the separation
  term's rank-2 update is also built here.
- As each group of 16 local expert centroids completes, its slice of the
  separation term (16 local E rows x all 256 V columns) is computed:
  two fp32 matmuls (-2 E^T V plus an aug rank-2 norms update), then an
  ACT sqrt/relu/square chain with accum_out.  Only the last group's
  ~1.5us of this is on the critical path.
- The expert clustering term uses the same identity; each core only
  contributes per-partition partials of sum(e), sum(e^2).

Tail (critical path after the last DMA): one AllGather of a [128, 3]
payload (sum_loc e, sum_loc e^2, local separation partials), a local
8-way reduce, the identity combine, and a final partition-fold matmul.
~12us vs ~35us for the previous gather-everything-then-pairwise scheme.

Measured on trn2 (8 cores, shared/noisy axon device) via in-kernel
For_i repeat loops at r=129 with round-robin min-delta timing: this
bulk phase runs within ~2-8us/iter of a DMA-only floor probe (same
descriptor stream, no reduction), where the floor itself sits at the
HBM roofline (64 MiB/core / ~358-410 GB/s per-core HBM port = ~160-190us
depending on device load).  The previous gather-everything baseline's
bulk measured +14-23us/iter over the floor in the same harness; adding
a 3rd DMA stream via gpsimd SWDGE cost ~+48us/iter (Q7 descriptor
generation can't keep up); dve_stop/bufs variations are within noise.
End-to-end: bulk + ~12us tail (last E group's separation rows ~1.5us,
payload AllGather at the HW-measured ~4.6us 8-core floor, reload +
reduce + combine + out DMA ~5us).
"""

import numpy as np

import concourse.bacc as bacc
import concourse.bass as bass
import concourse.mybir as mybir
import concourse.tile as tile
from concourse.bass_utils import run_bass_kernel_spmd
from concourse.tile import add_dep_helper

F32 = mybir.dt.float32
AF = mybir.ActivationFunctionType

MARGIN = 10.0
ALPHA = 3.0
BETA = 0.3
GAMMA = 0.3

B, S, D = 256, 2048, 128
N_CORES = 8
BUFS = 8     # big-tile pool buffers
GRP = 16     # batch columns per PSUM accumulation group
DVE_STOP = 128  # halving-tree handoff width (elems); PE folds the rest
N_DMA_ENG = 2   # DMA issue streams: 2 = SP+ACT HWDGE, 3 = + gpsimd SWDGE


def _build_body(tc, e, v, out, loc_v, gath_v, loc_fin, gath_fin, B, S, D,
                n_cores, solo=False, bufs=BUFS, loop_r=1, dve_stop=None,
                n_dma_eng=None, interleave=True, probe_dma_only=False):
    nc = tc.nc
    if dve_stop is None:
        dve_stop = DVE_STOP
    if n_dma_eng is None:
        n_dma_eng = N_DMA_ENG
    Bl = B // n_cores  # local batches per tensor
    J = S // 128       # seq tiles per batch item
    n_pairs = B * (B - 1) // 2
    w_ev = ALPHA / (B * B)
    w_ee = BETA / n_pairs
    w_vv = GAMMA / n_pairs
    n_groups = 2 * Bl // GRP          # 4: groups 0..1 violator, 2..3 expert
    n_vg = Bl // GRP                  # groups holding violator batches
    n_eg = n_groups - n_vg            # groups holding expert batches
    assert n_vg * GRP == Bl

    from contextlib import ExitStack

    with ExitStack() as ctx:
        consts = ctx.enter_context(tc.tile_pool(name="consts", bufs=1))
        # preamble-initialized const (no Tile dep => no extra sem wait on
        # matmuls; walrus allows only 1 sync wait per fp32 matmul)
        ones_col = nc.const_aps.aps[(F32, 1.0)]
        b_eps = consts.tile([128, 1], F32, name="b_eps")
        nc.vector.memset(b_eps[:], 1e-12)
        b_margin = consts.tile([128, 1], F32, name="b_margin")
        nc.vector.memset(b_margin[:], MARGIN)
        centS = consts.tile([D, 2 * Bl], F32, name="centS")

        sp = ctx.enter_context(tc.tile_pool(name="sp", bufs=1))
        CtV = sp.tile([D, B], F32, name="CtV")       # all violator centroids
        sqV = sp.tile([D, B], F32, name="sqV")
        rhs_v = sp.tile([64, B], F32, name="rhs_v")  # rows: 1 (r0), n_v (r32)
        ag_e = sp.tile([64, Bl], F32, name="ag_e")   # rows: n_e (r0), 1 (r32)
        m2E = sp.tile([D, Bl], F32, name="m2E")      # -2 * local E centroids
        sqEl = sp.tile([D, Bl], F32, name="sqEl")    # local E centroids ^2
        vvcol = sp.tile([128, 1], F32, name="vvcol")  # per-d VV partials
        vv_vec = sp.tile([128, 1], F32, name="vv_vec")
        vv_sq = sp.tile([128, 1], F32, name="vv_sq")
        acc_ev = sp.tile([128, n_eg], F32, name="acc_ev")
        payload = sp.tile([128, 3], F32, name="payload")
        G8r = sp.tile([128, 8 * 3], F32, name="G8r")
        trA = sp.tile([128, B], F32, name="trA")     # scratch
        trB = sp.tile([128, B], F32, name="trB")
        trC = sp.tile([128, B], F32, name="trC")
        tot = sp.tile([128, 1], F32, name="tot")
        fin = sp.tile([1, 1], F32, name="fin")
        nc.vector.memset(acc_ev[:], 0.0)
        nc.vector.memset(ag_e[:], 0.0)
        nc.vector.memset(ag_e[32:33, :], 1.0)
        nc.vector.memset(rhs_v[:], 0.0)
        nc.vector.memset(rhs_v[0:1, :], 1.0)
        if probe_dma_only:
            nc.vector.memset(centS[:], 0.0)

        dma_engines = [nc.sync, nc.scalar, nc.gpsimd][:n_dma_eng]
        cent_copies = []
        pe_gate = [None]  # PE nop absorbing CtV/rhs_v deps for walrus

        def emit_v_phase(pair_pool):
            # centS V-half -> dram -> AllGather -> CtV [D, 256]; then the
            # VV clustering identity + the n_v aug row.  All of this hides
            # under the expert-half DMA stream.
            nc.gpsimd.dma_start(out=loc_v[:], in_=centS[:, 0:Bl])
            if solo:
                nc.gpsimd.dma_start(out=gath_v[:], in_=loc_v[:])
            else:
                nc.gpsimd.collective_compute(
                    "AllGather",
                    mybir.AluOpType.bypass,
                    replica_groups=[list(range(n_cores))],
                    ins=[loc_v[:]],
                    outs=[gath_v[:]],
                )
            ct_dma = nc.gpsimd.dma_start(
                out=CtV[:].rearrange("d (c b) -> d c b", c=n_cores),
                in_=gath_v.rearrange("(c d) b -> d c b", c=n_cores),
            )
            # sqV + per-d sum_j v_jd^2 in one ACT op; per-d sum_j v_jd
            nc.scalar.activation(sqV[:], CtV[:], AF.Square,
                                 accum_out=vv_sq[:])
            nc.scalar.activation(trA[:], CtV[:], AF.Copy,
                                 accum_out=vv_vec[:])
            nc.vector.tensor_mul(trB[:, 0:1], vv_vec[:], vv_vec[:])
            nc.vector.scalar_tensor_tensor(
                vvcol[:], vv_sq[:], float(B), trB[:, 0:1],
                op0=mybir.AluOpType.mult, op1=mybir.AluOpType.subtract,
            )
            # n_v row at partition 32 (built there by the matmul itself;
            # engines cannot copy across partitions)
            psnv = pair_pool.tile([128, 512], F32, name="psnv", tag="pairps")
            nc.tensor.matmul(out=psnv[32:33, 0:B], lhsT=ones_col, rhs=sqV[:])
            rv_cp = nc.vector.tensor_copy(rhs_v[32:33, :], psnv[32:33, 0:B])
            # gate: later fp32 matmuls reach CtV/rhs_v through PE program
            # order instead of extra sem waits
            gate = nc.tensor.nop()
            add_dep_helper(gate.ins, ct_dma.ins, sync=True, reason="gate ctv")
            add_dep_helper(gate.ins, rv_cp.ins, sync=True, reason="gate rhsv")
            pe_gate[0] = gate

        def emit_e_pairwise(k, pair_pool, trash_pool):
            # separation rows for local-E group k: 16 rows x all 256 V
            c0, c1 = k * GRP, (k + 1) * GRP
            cent = centS[:, Bl + c0 : Bl + c1]
            nc.vector.tensor_scalar_mul(m2E[:, c0:c1], cent, -2.0)
            nc.vector.tensor_mul(sqEl[:, c0:c1], cent, cent)
            psn = pair_pool.tile([128, 512], F32, name="psn", tag="pairps")
            nc.tensor.matmul(out=psn[0:1, 0:GRP], lhsT=ones_col,
                             rhs=sqEl[:, c0:c1])
            nc.scalar.copy(ag_e[0:1, c0:c1], psn[0:1, 0:GRP])
            P_t = pair_pool.tile([128, 512], F32, name="P_t", tag="pairps")
            P = P_t[0:GRP, 0:B]
            mm1 = nc.tensor.matmul(out=P, lhsT=m2E[:, c0:c1], rhs=CtV[:],
                                   start=True, stop=False)
            mm2 = nc.tensor.matmul(out=P, lhsT=ag_e[:, c0:c1], rhs=rhs_v[:],
                                   start=False, stop=True)
            if pe_gate[0] is not None:
                add_dep_helper(mm1.ins, pe_gate[0].ins, sync=False,
                               reason="after gate")
                add_dep_helper(mm2.ins, pe_gate[0].ins, sync=False,
                               reason="after gate")
            dist = trash_pool.tile([128, B], F32, name="dist")
            hin = trash_pool.tile([128, B], F32, name="hin")
            hsq = trash_pool.tile([128, B], F32, name="hsq")
            nc.vector.tensor_scalar_max(P, P, 0.0)
            nc.scalar.activation(dist[:GRP], P, AF.Sqrt, bias=b_eps[:GRP])
            nc.scalar.activation(hin[:GRP], dist[:GRP], AF.Relu,
                                 bias=b_margin[:GRP], scale=-1.0)
            nc.scalar.activation(hsq[:GRP], hin[:GRP], AF.Square,
                                 accum_out=acc_ev[0:GRP, k : k + 1])

        # ---- bulk phase: per-batch centroid sums via TensorE ----
        with (
            tc.tile_pool(name="big", bufs=bufs) as big_pool,
            tc.tile_pool(name="cps", bufs=n_groups, space="PSUM") as cps,
            tc.tile_pool(name="pair", bufs=8 - n_groups, space="PSUM") as prp,
            tc.tile_pool(name="trash", bufs=2) as trash_pool,
        ):
            def emit_bulk(interleave_now):
                dma_i = 0
                for g in range(n_groups):
                    G = cps.tile([128, 512], F32, name="Gacc")
                    start_mm = None
                    col_last = []
                    for c in range(GRP):
                        gi0 = g * GRP + c
                        t_idx, b0 = divmod(gi0, Bl)
                        src = (v, e)[t_idx]  # violator batches first
                        Tb = big_pool.tile([128, J * D], F32, name="Tb")
                        eng = dma_engines[dma_i % len(dma_engines)]
                        dma_i += 1
                        eng.dma_start(
                            out=Tb[:],
                            in_=src[b0].rearrange("(p j) d -> p (j d)", p=128),
                        )
                        if probe_dma_only:
                            continue
                        # tree-halve the 16 seq rows per partition on DVE
                        # (exact fp32 adds) down to width dve_stop
                        w = J * D // 2
                        while w >= dve_stop:
                            nc.vector.tensor_add(
                                Tb[:, 0:w], Tb[:, 0:w], Tb[:, w : 2 * w]
                            )
                            w //= 2
                        n_folds = dve_stop // D
                        for fi in range(n_folds):
                            is_first = c == 0 and fi == 0
                            is_last = c == GRP - 1 and fi == n_folds - 1
                            o = fi * D
                            mm = nc.tensor.matmul(
                                out=G[:, c : c + 1],
                                lhsT=Tb[:, o : o + D],
                                rhs=ones_col,
                                start=is_first,
                                stop=is_last,
                            )
                            if start_mm is None:
                                start_mm = mm
                            elif fi == 0:
                                add_dep_helper(
                                    mm.ins, start_mm.ins, sync=False,
                                    reason="psum group start first",
                                )
                            if fi == n_folds - 1:
                                col_last.append(mm)
                            if is_last:
                                for prev in col_last[:-1]:
                                    add_dep_helper(
                                        mm.ins, prev.ins, sync=False,
                                        reason="psum group stop last",
                                    )
                    if not probe_dma_only:
                        cent_copies.append(
                            nc.scalar.mul(
                                centS[:, g * GRP : (g + 1) * GRP], G[:, 0:GRP],
                                1.0 / S,
                            )
                        )
                    if interleave_now:
                        # v-phase emitted after the first E group's folds so
                        # its PE ops don't queue-block them; execution still
                        # starts as soon as the V centroids exist.
                        if g == n_vg:
                            emit_v_phase(prp)
                        if g >= n_vg:
                            emit_e_pairwise(g - n_vg, prp, trash_pool)

            if loop_r > 1:
                with tc.For_i(0, loop_r, 1) as _i:
                    emit_bulk(False)
            else:
                emit_bulk(interleave)
            if not interleave or loop_r > 1:
                emit_v_phase(prp)
                for k in range(n_eg):
                    emit_e_pairwise(k, prp, trash_pool)

            # ---- tail: tiny payload AllGather + identity combine ----
            nc.scalar.activation(trA[:, 0:Bl], centS[:, Bl : 2 * Bl], AF.Copy,
                                 accum_out=payload[:, 0:1])
            nc.scalar.activation(trB[:, 0:Bl], sqEl[:], AF.Copy,
                                 accum_out=payload[:, 1:2])
            nc.vector.tensor_add(payload[:, 2:3], acc_ev[:, 0:1],
                                 acc_ev[:, 1:2])
            for k in range(2, n_eg):
                nc.vector.tensor_add(payload[:, 2:3], payload[:, 2:3],
                                     acc_ev[:, k : k + 1])
            nc.sync.dma_start(out=loc_fin[:], in_=payload[:])
            if solo:
                nc.sync.dma_start(out=gath_fin[:], in_=loc_fin[:])
            else:
                nc.gpsimd.collective_compute(
                    "AllGather",
                    mybir.AluOpType.bypass,
                    replica_groups=[list(range(n_cores))],
                    ins=[loc_fin[:]],
                    outs=[gath_fin[:]],
                )
            nc.sync.dma_start(
                out=G8r[:, 0 : 3 * n_cores].rearrange(
                    "p (c k) -> p c k", c=n_cores
                ),
                in_=gath_fin.rearrange("(c p) k -> p c k", c=n_cores),
            )
            # 8-way reduce of the gathered payloads -> Ssum [128, 3]
            red = G8r[:, 0 : 3 * n_cores]
            if n_cores == 8:
                nc.vector.tensor_add(trA[:, 0:12], red[:, 0:12], red[:, 12:24])
                nc.vector.tensor_add(trB[:, 0:6], trA[:, 0:6], trA[:, 6:12])
                nc.vector.tensor_add(trC[:, 0:3], trB[:, 0:3], trB[:, 3:6])
                Ssum = trC[:, 0:3]
            else:
                assert n_cores == 1
                Ssum = red[:, 0:3]
            # EE identity: B * sum||e||^2 - ||sum e||^2 (per-d partials)
            nc.vector.tensor_mul(trA[:, 0:1], Ssum[:, 0:1], Ssum[:, 0:1])
            nc.vector.scalar_tensor_tensor(
                trB[:, 0:1], Ssum[:, 1:2], float(B), trA[:, 0:1],
                op0=mybir.AluOpType.mult, op1=mybir.AluOpType.subtract,
            )
            nc.vector.tensor_scalar_mul(tot[:], Ssum[:, 2:3], w_ev)
            nc.vector.scalar_tensor_tensor(
                tot[:], trB[:, 0:1], w_ee, tot[:],
                op0=mybir.AluOpType.mult, op1=mybir.AluOpType.add,
            )
            nc.vector.scalar_tensor_tensor(
                tot[:], vvcol[:], w_vv, tot[:],
                op0=mybir.AluOpType.mult, op1=mybir.AluOpType.add,
            )
            psF_t = prp.tile([128, 512], F32, name="psF", tag="pairps")
            psF = psF_t[0:1, 0:1]
            nc.tensor.matmul(out=psF, lhsT=ones_col, rhs=tot[:])
            nc.scalar.copy(fin[:], psF)
            nc.sync.dma_start(out=out[:], in_=fin[:])


def build_nc(B=B, S=S, D=D, n_cores=N_CORES, solo=False, bufs=None,
             loop_r=1, dve_stop=None, n_dma_eng=None, interleave=True,
             probe_dma_only=False):
    Bl = B // n_cores
    nc = bacc.Bacc("TRN2", num_devices=n_cores)
    e = nc.dram_tensor("expert_concepts", [Bl, S, D], F32,
                       kind="ExternalInput").ap()
    v = nc.dram_tensor("violator_concepts", [Bl, S, D], F32,
                       kind="ExternalInput").ap()
    out = nc.dram_tensor("out", [1, 1], F32, kind="ExternalOutput").ap()
    loc_v = nc.dram_tensor("loc_v", [D, Bl], F32).ap()
    loc_fin = nc.dram_tensor("loc_fin", [128, 3], F32).ap()
    gath_space = "Local" if solo else "Shared"
    gath_v = nc.dram_tensor(
        "gath_v", [n_cores * D, Bl], F32, addr_space=gath_space
    ).ap()
    gath_fin = nc.dram_tensor(
        "gath_fin", [n_cores * 128, 3], F32, addr_space=gath_space
    ).ap()
    with tile.TileContext(nc) as tc:
        _build_body(
            tc, e, v, out, loc_v, gath_v, loc_fin, gath_fin, B, S, D,
            n_cores, solo=solo,
            bufs=bufs if bufs is not None else BUFS,
            loop_r=loop_r,
            dve_stop=dve_stop if dve_stop is not None else DVE_STOP,
            n_dma_eng=n_dma_eng,
            interleave=interleave,
            probe_dma_only=probe_dma_only,
        )
    nc.compile()
    return nc


def _run(expert_concepts, violator_concepts, **spmd_kwargs):
    expert_concepts = np.ascontiguousarray(expert_concepts, dtype=np.float32)
    violator_concepts = np.ascontiguousarray(violator_concepts, dtype=np.float32)
    assert expert_concepts.shape == (B, S, D)
    assert violator_concepts.shape == (B, S, D)

    nc = build_nc()
    Bl = B // N_CORES
    in_maps = [
        {
            "expert_concepts": expert_concepts[c * Bl : (c + 1) * Bl],
            "violator_concepts": violator_concepts[c * Bl : (c + 1) * Bl],
        }
        for c in range(N_CORES)
    ]
    res = run_bass_kernel_spmd(nc, in_maps, list(range(N_CORES)), **spmd_kwargs)
    return np.float32(res.results[0]["out"][0, 0]), res


def kernel(expert_concepts: np.ndarray, violator_concepts: np.ndarray) -> np.ndarray:
    out, _ = _run(expert_concepts, violator_concepts)
    return out
